# revision 38
# baseline (speedup 1.0000x reference)
"""MMD loss kernel for Trainium2 (8 NeuronCores, Bass/Tile).

Math: out = mean_k mean_ij exp(-c_k * ||x_i - x_j||^2)          (kss)
          + same for y                                          (ktt)
          - 2 * same for (x, y)                                 (kst)
      with c_k = 1/(2 b_k^2), x: [8192, 256], y: [8192, 256].

Device strategy (identical SPMD program on 8 cores, different data):
  * PE computes the pairwise squared distances directly via feature
    augmentation: dist = [-2x; nh; nl; 1; 1]^T . [y; 1; 1; nh; nl]
    in bf16 (fp32 PSUM accumulate), K = 256 + 4.
  * ScalarE evaluates exp(-c_k * d) straight from PSUM in [128, 2048]
    chunks with fused accum_out row-sums (the mean reduction is free).
  * kss/ktt use a symmetric band decomposition: each 128-row tile r
    covers col tiles r+1..r+32 (mod 64) with weight 2, a d=32 batch
    with weight -1 removes the double count, and the diagonal subtiles
    (weight +1) have their exact diagonal masked to +1e30 (exp -> 0);
    the true diagonal contribution (N*K per matrix) is added on the
    host analytically.  This removes 1/3 of the exp work.
  * Per-core work: row tiles {8j + core}.  A per-core column rotation
    by 128*(core+1) makes every access offset core-independent, so one
    NEFF serves all 8 cores.
  * Host: builds bf16 operands, runs the NEFF on cores 0-7, and
    combines the per-chunk accumulator columns with the chunk weights.
"""

import os
import numpy as np
import ml_dtypes

import concourse.bass as bass
import concourse.mybir as mybir
import concourse.tile as tile
from concourse import bacc
from concourse.bass_utils import run_bass_kernel_spmd

bf16 = ml_dtypes.bfloat16

N, D, P = 8192, 256, 128
NCORES, JPC = 8, 8          # 64 row tiles of 128, 8 per core
CHUNK = 2048                # PSUM chunk (4 banks) / ACT free dim
BANK = 512
NT = N // P                 # 64 subtile columns
BIG = np.float32(1e30)

# ---------------------------------------------------------------- job list


def chunk_list():
    """Chunk descriptors, identical on every core.

    (kind, lhs_tile, rhs_role, rhs_start, weight)
      kind: 'mm' (12-matmul streaming chunk) or 'sub16' (16 subtiles)
    """
    chunks = []
    # kst column-major: the 8 jobs of column piece cb only need that piece
    # of ry, so compute starts as soon as the first ~1 MB of DMA lands.
    for cb in range(4):
        for j in range(JPC):                  # kst, weight -2
            chunks.append(("mm", j, "y", cb * CHUNK, -2.0, False))
    for j in range(JPC):                      # kss band, weight +2
        for cb in range(2):
            chunks.append(("mm", j, "x", (1024 * j + CHUNK * cb) % N, 2.0, False))
    # the sub16 specials sit mid-stream so the kernel tail stays on the
    # regular pipeline (their DVE chains are unpaired and would trail)
    chunks.append(("sub16", None, None, "d32", -1.0, False))   # d=32 fix
    chunks.append(("sub16", None, None, "diag", 1.0, True))    # masked diag
    for j in range(JPC):                      # ktt band, weight +2
        for cb in range(2):
            chunks.append(("mm", 8 + j, "y", (1024 * j + CHUNK * cb) % N, 2.0, False))
    return chunks


def sub16_layout(batch):
    """16 (lhs_tile, role, rhs_start) triples for a sub16 chunk."""
    out = []
    for s in range(16):
        jj = s % 8
        role = "x" if s < 8 else "y"
        if batch == "d32":
            st = (1024 * jj + 3968) % N
        else:
            st = (1024 * jj - 128) % N
        out.append((s, role, st))
    return out


NCHUNKS = len(chunk_list())  # 66

# ---------------------------------------------------------------- device


def pick_split(cs):
    """Find power-of-4 chains so some exp terms move to VectorE.

    Returns (base_idx, pow4_idx, pow16_idx|None) or None.  For the
    canonical bandwidths [0.1, 0.5, 1, 2, 5] -> cs = [50, 2, .5, .125,
    .02]: base c=0.125 (b=2), offloaded c=0.5 = base^4 and c=2 = base^16.
    """
    K = len(cs)

    def near(a, b):
        return abs(a - b) <= 1e-6 * abs(b)

    best = None
    for i in range(K):
        for j in range(K):
            if i == j or not near(cs[j], 4.0 * cs[i]):
                continue
            if best is None:
                best = (i, j, None)
            for k in range(K):
                if k not in (i, j) and near(cs[k], 4.0 * cs[j]):
                    return (i, j, k)
    return best





def build_kernel(neg_cs, split=None):
    """Build + compile the SPMD NEFF for the given -c_k immediates."""
    K = len(neg_cs)
    nc = bacc.Bacc("TRN2", debug=False, enable_asserts=False, num_devices=NCORES)
    f32, b16 = mybir.dt.float32, mybir.dt.bfloat16

    d_lhs0 = nc.dram_tensor("lhs0", [P, 16 * P], b16, kind="ExternalInput").ap()
    d_lhs1 = nc.dram_tensor("lhs1", [P, 16 * P], b16, kind="ExternalInput").ap()
    d_laug = nc.dram_tensor("laug", [4, 16 * P], b16, kind="ExternalInput").ap()
    d_rx0 = nc.dram_tensor("rx0", [P, N], b16, kind="ExternalInput").ap()
    d_rx1 = nc.dram_tensor("rx1", [P, N], b16, kind="ExternalInput").ap()
    d_ry0 = nc.dram_tensor("ry0", [P, N], b16, kind="ExternalInput").ap()
    d_ry1 = nc.dram_tensor("ry1", [P, N], b16, kind="ExternalInput").ap()
    d_raugx = nc.dram_tensor("raugx", [4, N], b16, kind="ExternalInput").ap()
    d_raugy = nc.dram_tensor("raugy", [4, N], b16, kind="ExternalInput").ap()
    d_mask = nc.dram_tensor("maskd", [P, CHUNK], b16, kind="ExternalInput").ap()
    d_acc = nc.dram_tensor("acc", [P, NCHUNKS * K], f32, kind="ExternalOutput").ap()

    with tile.TileContext(nc) as tc:
        with (
            tc.tile_pool(name="consts", bufs=1) as consts,
            tc.tile_pool(name="scr", bufs=2) as scrp,
            tc.tile_pool(name="psum", bufs=2, space="PSUM") as psump,
        ):
            lhs0 = consts.tile([P, 16 * P], b16)
            lhs1 = consts.tile([P, 16 * P], b16)
            laug = consts.tile([4, 16 * P], b16)
            rx0 = consts.tile([P, N], b16)
            rx1 = consts.tile([P, N], b16)
            ry0 = consts.tile([P, N], b16)
            ry1 = consts.tile([P, N], b16)
            raugx = consts.tile([4, N], b16)
            raugy = consts.tile([4, N], b16)
            maskd = consts.tile([P, CHUNK], b16)
            acc = consts.tile([P, NCHUNKS * K], f32)

            nc.vector.memset(acc, 0.0)
            # DMA order matters: the first kst chunks need the x-role lhs
            # tiles + aug + the first ry column pieces; everything else
            # streams underneath the first chunks' compute.
            half = 8 * P
            for sb, dr in ((lhs0, d_lhs0), (lhs1, d_lhs1), (laug, d_laug)):
                nc.sync.dma_start(out=sb[:, :half], in_=dr[:, :half])
            nc.sync.dma_start(out=raugy, in_=d_raugy)
            for sb, dr in ((lhs0, d_lhs0), (lhs1, d_lhs1), (laug, d_laug)):
                nc.sync.dma_start(out=sb[:, half:], in_=dr[:, half:])
            nc.sync.dma_start(out=raugx, in_=d_raugx)
            for piece in range(4):
                csl = slice(CHUNK * piece, CHUNK * (piece + 1))
                for sb, dr in ((ry0, d_ry0), (ry1, d_ry1)):
                    nc.sync.dma_start(out=sb[:, csl], in_=dr[:, csl])
            for piece in range(4):
                csl = slice(CHUNK * piece, CHUNK * (piece + 1))
                for sb, dr in ((rx0, d_rx0), (rx1, d_rx1)):
                    nc.sync.dma_start(out=sb[:, csl], in_=dr[:, csl])
            nc.sync.dma_start(out=maskd, in_=d_mask)

            rmain = {"x": (rx0, rx1), "y": (ry0, ry1)}
            raug_t = {"x": raugx, "y": raugy}

            def emit_chunk_mms(psum, jobs):
                """jobs: list of (pcol, width, lhs_tile, role, rhs_start).
                k-outer / job-inner order so each lhsT loads once per
                contraction slice instead of once per bank."""
                for ki in range(3):
                    for (pcol, width, t, role, start) in jobs:
                        m0, m1 = rmain[role]
                        lsl = slice(P * t, P * t + P)
                        if ki == 0:
                            l, r = lhs0[:, lsl], m0[:, start : start + width]
                        elif ki == 1:
                            l, r = lhs1[:, lsl], m1[:, start : start + width]
                        else:
                            l, r = (
                                laug[:, lsl],
                                raug_t[role][:, start : start + width],
                            )
                        nc.tensor.matmul(
                            psum[:, pcol : pcol + width], l, r,
                            start=(ki == 0), stop=(ki == 2),
                        )

            def emit_dve_chain(base_ap, cols, slot_j, slot_k):
                """Power chain on VectorE over base_ap [P, cols]:
                t4 = base^4 (sum -> slot_j), t16 = base^16 (sum -> slot_k).
                (tensor_tensor_reduce crashes the NEFF on this HW path;
                scalar_tensor_tensor's accum_out works.)"""
                t2 = scrp.tile([P, 2 * CHUNK], b16, tag="tmp", name="tmp")[:, :cols]
                t4 = scrp.tile([P, 2 * CHUNK], b16, tag="t4", name="t4")[:, :cols]
                nc.vector.tensor_mul(t2, base_ap, base_ap)
                nc.vector.scalar_tensor_tensor(
                    out=t4, in0=t2, scalar=1.0, in1=t2,
                    op0=mybir.AluOpType.mult, op1=mybir.AluOpType.mult,
                    accum_out=slot_j,
                )
                if slot_k is not None:
                    t8 = scrp.tile([P, 2 * CHUNK], b16, tag="tmp", name="tmp")[:, :cols]
                    scr = scrp.tile([P, 2 * CHUNK], b16, tag="scr", name="scr")[:, :cols]
                    nc.vector.tensor_mul(t8, t4, t4)
                    nc.vector.scalar_tensor_tensor(
                        out=scr, in0=t8, scalar=1.0, in1=t8,
                        op0=mybir.AluOpType.mult, op1=mybir.AluOpType.mult,
                        accum_out=slot_k,
                    )

            chunks = chunk_list()
            pend = None  # (base_pair_tile, even_q) awaiting its odd partner
            for q, (kind, t, role, start, _w, mask) in enumerate(chunks):
                psum = psump.tile([P, CHUNK], f32)
                if kind == "mm":
                    jobs = [
                        (BANK * b, BANK, t, role, (start + BANK * b) % N)
                        for b in range(4)
                    ]
                else:
                    jobs = [
                        (P * s16, P, s16, role2, st2)
                        for (s16, role2, st2) in sub16_layout(start)
                    ]
                emit_chunk_mms(psum, jobs)
                if mask:
                    nc.vector.tensor_add(psum, psum, maskd)
                if split is None:
                    scr2 = scrp.tile([P, 2 * CHUNK], b16, tag="scr2")
                    for k, ncs in enumerate(neg_cs):
                        nc.scalar.activation(
                            out=scr2[:, :CHUNK],
                            in_=psum,
                            func=mybir.ActivationFunctionType.Exp,
                            scale=float(ncs),
                            accum_out=acc[:, q * K + k : q * K + k + 1],
                        )
                    continue

                bi, pj, pk = split
                if q >= len(chunks) - 2:
                    # tail rebalance: ACT takes c_k back so VectorE's
                    # trailing chains don't outlive the last ACT work
                    pk = None
                # Pair adjacent same-weight chunks: both bases land in one
                # [P, 4096] tile and the DVE chain runs once at FD 4096.
                # Its accum goes to the even chunk's slots; the odd slots
                # stay zero (memset) so the host weighting is unchanged.
                can_pair = (
                    kind == "mm"
                    and pend is None
                    and q + 1 < len(chunks)
                    and chunks[q + 1][0] == "mm"
                    and chunks[q + 1][4] == _w
                )
                if can_pair:
                    bpair = scrp.tile([P, 2 * CHUNK], b16, tag="base")
                    half = slice(0, CHUNK)
                    pend = (bpair, q)
                elif pend is not None:
                    bpair = pend[0]
                    half = slice(CHUNK, 2 * CHUNK)
                else:
                    bpair = scrp.tile([P, 2 * CHUNK], b16, tag="base")
                    half = slice(0, CHUNK)
                # base term first so DVE can overlap the rest
                nc.scalar.activation(
                    out=bpair[:, half], in_=psum,
                    func=mybir.ActivationFunctionType.Exp,
                    scale=float(neg_cs[bi]),
                    accum_out=acc[:, q * K + bi : q * K + bi + 1],
                )
                if can_pair:
                    pass  # chain emitted after the partner chunk's base
                elif pend is not None:
                    q0 = pend[1]
                    emit_dve_chain(
                        bpair[:, : 2 * CHUNK], 2 * CHUNK,
                        acc[:, q0 * K + pj : q0 * K + pj + 1],
                        None if pk is None
                        else acc[:, q0 * K + pk : q0 * K + pk + 1],
                    )
                    pend = None
                else:
                    emit_dve_chain(
                        bpair[:, :CHUNK], CHUNK,
                        acc[:, q * K + pj : q * K + pj + 1],
                        None if pk is None
                        else acc[:, q * K + pk : q * K + pk + 1],
                    )
                scr2 = scrp.tile([P, 2 * CHUNK], b16, tag="scr2")
                for k, ncs in enumerate(neg_cs):
                    if k in (bi, pj, pk):
                        continue
                    nc.scalar.activation(
                        out=scr2[:, :CHUNK], in_=psum,
                        func=mybir.ActivationFunctionType.Exp,
                        scale=float(ncs),
                        accum_out=acc[:, q * K + k : q * K + k + 1],
                    )

            nc.sync.dma_start(out=d_acc, in_=acc)

    nc.compile()
    return nc


# ---------------------------------------------------------------- host


def _split_hi_lo(v64):
    hi = v64.astype(bf16)
    lo = (v64 - hi.astype(np.float64)).astype(bf16)
    return hi, lo


def _build_core_inputs(xT_b, yT_b, xnorm, ynorm, core):
    """Per-core input dict. xT_b/yT_b: [D, N] bf16; norms f64 [N]."""
    shift = P * (core + 1)
    rx = np.roll(xT_b, -shift, axis=1)
    ry = np.roll(yT_b, -shift, axis=1)
    ones = np.ones(N, bf16)
    xh, xl = _split_hi_lo(np.roll(xnorm, -shift))
    yh, yl = _split_hi_lo(np.roll(ynorm, -shift))
    raugx = np.stack([ones, ones, xh, xl])
    raugy = np.stack([ones, ones, yh, yl])

    lhs = np.empty((D, 16 * P), bf16)
    laug = np.empty((4, 16 * P), bf16)
    one128 = np.ones(P, bf16)
    for t in range(16):
        r = 8 * (t % 8) + core
        rows = slice(P * r, P * r + P)
        src = xT_b if t < 8 else yT_b
        nsrc = xnorm if t < 8 else ynorm
        lhs[:, P * t : P * (t + 1)] = (
            -2.0 * src[:, rows].astype(np.float32)
        ).astype(bf16)
        nh, nl = _split_hi_lo(nsrc[rows])
        laug[:, P * t : P * (t + 1)] = np.stack([nh, nl, one128, one128])

    mask = np.zeros((P, CHUNK), bf16)
    for s in range(16):
        mask[np.arange(P), P * s + np.arange(P)] = bf16(BIG)

    return {
        "lhs0": np.ascontiguousarray(lhs[:P]),
        "lhs1": np.ascontiguousarray(lhs[P:]),
        "laug": np.ascontiguousarray(laug),
        "rx0": np.ascontiguousarray(rx[:P]),
        "rx1": np.ascontiguousarray(rx[P:]),
        "ry0": np.ascontiguousarray(ry[:P]),
        "ry1": np.ascontiguousarray(ry[P:]),
        "raugx": np.ascontiguousarray(raugx),
        "raugy": np.ascontiguousarray(raugy),
        "maskd": mask,
    }


_NC_CACHE = {}
_WARM = [False]


def _warmup():
    """Run a trivial NEFF once per process: the first NEFF execution in
    an axon session pays ~95 us of ring/queue init that would otherwise
    land inside the measured kernel."""
    if _WARM[0]:
        return
    nc = bacc.Bacc("TRN2", debug=False, enable_asserts=False, num_devices=NCORES)
    f32 = mybir.dt.float32
    d_in = nc.dram_tensor("wx", [P, P], f32, kind="ExternalInput").ap()
    d_out = nc.dram_tensor("wy", [P, P], f32, kind="ExternalOutput").ap()
    with tile.TileContext(nc) as tc:
        with tc.tile_pool(name="pool", bufs=1) as pool:
            t = pool.tile([P, P], f32)
            nc.sync.dma_start(out=t, in_=d_in)
            nc.sync.dma_start(out=d_out, in_=t)
    nc.compile()
    x = np.zeros((P, P), np.float32)
    for attempt in range(3):
        try:
            run_bass_kernel_spmd(
                nc, [{"wx": x}] * NCORES, core_ids=list(range(NCORES))
            )
            break
        except Exception:
            if attempt == 2:
                raise
            import time

            time.sleep(10)
    _WARM[0] = True


def _get_kernel(neg_cs, use_split=True):
    split = pick_split([-v for v in neg_cs]) if use_split else None
    key = (tuple(float(v) for v in neg_cs), split)
    if key not in _NC_CACHE:
        _NC_CACHE[key] = build_kernel(neg_cs, split=split)
    return _NC_CACHE[key]


def _run(source_features, target_features, bandwidths, trace=False, use_split=True):
    x = np.asarray(source_features, np.float32)
    y = np.asarray(target_features, np.float32)
    b = np.asarray(bandwidths, np.float64)
    cs = 1.0 / (2.0 * b * b)
    K = len(cs)
    neg_cs = [-float(c) for c in cs]

    xT_b = np.ascontiguousarray(x.T).astype(bf16)
    yT_b = np.ascontiguousarray(y.T).astype(bf16)
    xnorm = (x.astype(np.float64) ** 2).sum(1)
    ynorm = (y.astype(np.float64) ** 2).sum(1)

    nc = _get_kernel(neg_cs, use_split=use_split)
    in_maps = [
        _build_core_inputs(xT_b, yT_b, xnorm, ynorm, c) for c in range(NCORES)
    ]
    _warmup()
    res = None
    for attempt in range(3):
        try:
            res = run_bass_kernel_spmd(
                nc, in_maps, core_ids=list(range(NCORES)), trace=trace
            )
            break
        except Exception:
            # transient device wedge (NRT_EXEC_UNIT_UNRECOVERABLE) clears
            # on a subsequent attempt; give it a moment and retry
            if attempt == 2:
                raise
            import time

            time.sleep(15)

    weights = np.array([w for (_, _, _, _, w, _) in chunk_list()], np.float64)
    total = 0.0
    for core in range(NCORES):
        a = res.results[core]["acc"].astype(np.float64)  # [P, NCHUNKS*K]
        per_chunk = a.sum(0).reshape(NCHUNKS, K).sum(1)
        total += float(per_chunk @ weights)
    total += 2.0 * N * K  # analytic masked diagonals of kss + ktt
    out = np.float32(total / (float(N) * float(N) * K))
    return np.array(out, dtype=np.float32), res


def kernel(source_features, target_features, bandwidths):
    out, _ = _run(source_features, target_features, bandwidths)
    return out


# revision 40
# speedup vs baseline: 1.1974x; 1.1974x over previous
"""MMD loss kernel for Trainium2 (8 NeuronCores, Bass/Tile).

Math: out = mean_k mean_ij exp(-c_k * ||x_i - x_j||^2)          (kss)
          + same for y                                          (ktt)
          - 2 * same for (x, y)                                 (kst)
      with c_k = 1/(2 b_k^2), x: [8192, 256], y: [8192, 256].

Device strategy (identical SPMD program on 8 cores, different data):
  * PE computes the pairwise squared distances directly via feature
    augmentation: dist = [-2x; nh; nl; 1; 1]^T . [y; 1; 1; nh; nl]
    in bf16 (fp32 PSUM accumulate), K = 256 + 4.
  * ScalarE evaluates exp(-c_k * d) straight from PSUM in [128, 2048]
    chunks with fused accum_out row-sums (the mean reduction is free).
  * kss/ktt use a symmetric band decomposition: each 128-row tile r
    covers col tiles r+1..r+32 (mod 64) with weight 2, a d=32 batch
    with weight -1 removes the double count, and the diagonal subtiles
    (weight +1) have their exact diagonal masked to +1e30 (exp -> 0);
    the true diagonal contribution (N*K per matrix) is added on the
    host analytically.  This removes 1/3 of the exp work.
  * Per-core work: row tiles {8j + core}.  A per-core column rotation
    by 128*(core+1) makes every access offset core-independent, so one
    NEFF serves all 8 cores.
  * Host: builds bf16 operands, runs the NEFF on cores 0-7, and
    combines the per-chunk accumulator columns with the chunk weights.
"""

import os
import numpy as np
import ml_dtypes

import concourse.bass as bass
import concourse.mybir as mybir
import concourse.tile as tile
from concourse import bacc
from concourse.bass_utils import run_bass_kernel_spmd

bf16 = ml_dtypes.bfloat16

N, D, P = 8192, 256, 128
NCORES, JPC = 8, 8          # 64 row tiles of 128, 8 per core
CHUNK = 2048                # PSUM chunk (4 banks) / ACT free dim
BANK = 512
NT = N // P                 # 64 subtile columns
BIG = np.float32(1e30)

# ---------------------------------------------------------------- job list


def chunk_list():
    """Chunk descriptors, identical on every core.

    (kind, lhs_tile, rhs_role, rhs_start, weight)
      kind: 'mm' (12-matmul streaming chunk) or 'sub16' (16 subtiles)
    """
    chunks = []
    # kst column-major: the 8 jobs of column piece cb only need that piece
    # of ry, so compute starts as soon as the first ~1 MB of DMA lands.
    for cb in range(4):
        for j in range(JPC):                  # kst, weight -2
            chunks.append(("mm", j, "y", cb * CHUNK, -2.0, False))
    for j in range(JPC):                      # kss band, weight +2
        for cb in range(2):
            chunks.append(("mm", j, "x", (1024 * j + CHUNK * cb) % N, 2.0, False))
    # the sub16 specials sit mid-stream so the kernel tail stays on the
    # regular pipeline (their DVE chains are unpaired and would trail)
    chunks.append(("sub16", None, None, "d32", -1.0, False))   # d=32 fix
    chunks.append(("sub16", None, None, "diag", 1.0, True))    # masked diag
    for j in range(JPC):                      # ktt band, weight +2
        for cb in range(2):
            chunks.append(("mm", 8 + j, "y", (1024 * j + CHUNK * cb) % N, 2.0, False))
    return chunks


def sub16_layout(batch):
    """16 (lhs_tile, role, rhs_start) triples for a sub16 chunk."""
    out = []
    for s in range(16):
        jj = s % 8
        role = "x" if s < 8 else "y"
        if batch == "d32":
            st = (1024 * jj + 3968) % N
        else:
            st = (1024 * jj - 128) % N
        out.append((s, role, st))
    return out


NCHUNKS = len(chunk_list())  # 66

# ---------------------------------------------------------------- device


def pick_split(cs):
    """Find power-of-4 chains so some exp terms move to VectorE.

    Returns (base_idx, pow4_idx, pow16_idx|None) or None.  For the
    canonical bandwidths [0.1, 0.5, 1, 2, 5] -> cs = [50, 2, .5, .125,
    .02]: base c=0.125 (b=2), offloaded c=0.5 = base^4 and c=2 = base^16.
    """
    K = len(cs)

    def near(a, b):
        return abs(a - b) <= 1e-6 * abs(b)

    best = None
    for i in range(K):
        for j in range(K):
            if i == j or not near(cs[j], 4.0 * cs[i]):
                continue
            if best is None:
                best = (i, j, None)
            for k in range(K):
                if k not in (i, j) and near(cs[k], 4.0 * cs[j]):
                    return (i, j, k)
    return best





def build_kernel(neg_cs, split=None):
    """Build + compile the SPMD NEFF for the given -c_k immediates."""
    K = len(neg_cs)
    nc = bacc.Bacc("TRN2", debug=False, enable_asserts=False, num_devices=NCORES)
    f32, b16 = mybir.dt.float32, mybir.dt.bfloat16

    d_lhs0 = nc.dram_tensor("lhs0", [P, 16 * P], b16, kind="ExternalInput").ap()
    d_lhs1 = nc.dram_tensor("lhs1", [P, 16 * P], b16, kind="ExternalInput").ap()
    d_laug = nc.dram_tensor("laug", [4, 16 * P], b16, kind="ExternalInput").ap()
    d_rx0 = nc.dram_tensor("rx0", [P, N], b16, kind="ExternalInput").ap()
    d_rx1 = nc.dram_tensor("rx1", [P, N], b16, kind="ExternalInput").ap()
    d_ry0 = nc.dram_tensor("ry0", [P, N], b16, kind="ExternalInput").ap()
    d_ry1 = nc.dram_tensor("ry1", [P, N], b16, kind="ExternalInput").ap()
    d_raugx = nc.dram_tensor("raugx", [4, N], b16, kind="ExternalInput").ap()
    d_raugy = nc.dram_tensor("raugy", [4, N], b16, kind="ExternalInput").ap()
    d_mask = nc.dram_tensor("maskd", [P, CHUNK], b16, kind="ExternalInput").ap()
    d_acc = nc.dram_tensor("acc", [P, NCHUNKS * K], f32, kind="ExternalOutput").ap()

    with tile.TileContext(nc) as tc:
        with (
            tc.tile_pool(name="consts", bufs=1) as consts,
            tc.tile_pool(name="scr", bufs=2) as scrp,
            tc.tile_pool(name="psum", bufs=2, space="PSUM") as psump,
        ):
            lhs0 = consts.tile([P, 16 * P], b16)
            lhs1 = consts.tile([P, 16 * P], b16)
            laug = consts.tile([4, 16 * P], b16)
            rx0 = consts.tile([P, N], b16)
            rx1 = consts.tile([P, N], b16)
            ry0 = consts.tile([P, N], b16)
            ry1 = consts.tile([P, N], b16)
            raugx = consts.tile([4, N], b16)
            raugy = consts.tile([4, N], b16)
            maskd = consts.tile([P, CHUNK], b16)
            acc = consts.tile([P, NCHUNKS * K], f32)

            nc.vector.memset(acc, 0.0)
            # DMA order matters: the first kst chunks need the x-role lhs
            # tiles + aug + the first ry column pieces; everything else
            # streams underneath the first chunks' compute.
            half = 8 * P
            for sb, dr in ((lhs0, d_lhs0), (lhs1, d_lhs1), (laug, d_laug)):
                nc.sync.dma_start(out=sb[:, :half], in_=dr[:, :half])
            nc.sync.dma_start(out=raugy, in_=d_raugy)
            for sb, dr in ((lhs0, d_lhs0), (lhs1, d_lhs1), (laug, d_laug)):
                nc.sync.dma_start(out=sb[:, half:], in_=dr[:, half:])
            nc.sync.dma_start(out=raugx, in_=d_raugx)
            for piece in range(4):
                csl = slice(CHUNK * piece, CHUNK * (piece + 1))
                for sb, dr in ((ry0, d_ry0), (ry1, d_ry1)):
                    nc.sync.dma_start(out=sb[:, csl], in_=dr[:, csl])
            for piece in range(4):
                csl = slice(CHUNK * piece, CHUNK * (piece + 1))
                for sb, dr in ((rx0, d_rx0), (rx1, d_rx1)):
                    nc.sync.dma_start(out=sb[:, csl], in_=dr[:, csl])
            nc.sync.dma_start(out=maskd, in_=d_mask)

            rmain = {"x": (rx0, rx1), "y": (ry0, ry1)}
            raug_t = {"x": raugx, "y": raugy}

            def emit_chunk_mms(psum, jobs):
                """jobs: list of (pcol, width, lhs_tile, role, rhs_start).
                k-outer / job-inner order so each lhsT loads once per
                contraction slice instead of once per bank."""
                for ki in range(3):
                    for (pcol, width, t, role, start) in jobs:
                        m0, m1 = rmain[role]
                        lsl = slice(P * t, P * t + P)
                        if ki == 0:
                            l, r = lhs0[:, lsl], m0[:, start : start + width]
                        elif ki == 1:
                            l, r = lhs1[:, lsl], m1[:, start : start + width]
                        else:
                            l, r = (
                                laug[:, lsl],
                                raug_t[role][:, start : start + width],
                            )
                        nc.tensor.matmul(
                            psum[:, pcol : pcol + width], l, r,
                            start=(ki == 0), stop=(ki == 2),
                        )

            def emit_dve_chain(base_ap, cols, slot_j, slot_k):
                """Power chain on VectorE over base_ap [P, cols]:
                t4 = base^4 (sum -> slot_j), t16 = base^16 (sum -> slot_k).
                (tensor_tensor_reduce crashes the NEFF on this HW path;
                scalar_tensor_tensor's accum_out works.)"""
                t2 = scrp.tile([P, 2 * CHUNK], b16, tag="tmp", name="tmp")[:, :cols]
                t4 = scrp.tile([P, 2 * CHUNK], b16, tag="t4", name="t4")[:, :cols]
                nc.vector.tensor_mul(t2, base_ap, base_ap)
                nc.vector.scalar_tensor_tensor(
                    out=t4, in0=t2, scalar=1.0, in1=t2,
                    op0=mybir.AluOpType.mult, op1=mybir.AluOpType.mult,
                    accum_out=slot_j,
                )
                if slot_k is not None:
                    t8 = scrp.tile([P, 2 * CHUNK], b16, tag="tmp", name="tmp")[:, :cols]
                    scr = scrp.tile([P, 2 * CHUNK], b16, tag="scr", name="scr")[:, :cols]
                    nc.vector.tensor_mul(t8, t4, t4)
                    nc.vector.scalar_tensor_tensor(
                        out=scr, in0=t8, scalar=1.0, in1=t8,
                        op0=mybir.AluOpType.mult, op1=mybir.AluOpType.mult,
                        accum_out=slot_k,
                    )

            chunks = chunk_list()
            pend = None  # (base_pair_tile, even_q) awaiting its odd partner
            for q, (kind, t, role, start, _w, mask) in enumerate(chunks):
                psum = psump.tile([P, CHUNK], f32)
                if kind == "mm":
                    jobs = [
                        (BANK * b, BANK, t, role, (start + BANK * b) % N)
                        for b in range(4)
                    ]
                else:
                    jobs = [
                        (P * s16, P, s16, role2, st2)
                        for (s16, role2, st2) in sub16_layout(start)
                    ]
                emit_chunk_mms(psum, jobs)
                if mask:
                    nc.vector.tensor_add(psum, psum, maskd)
                if split is None:
                    scr2 = scrp.tile([P, 2 * CHUNK], b16, tag="scr2")
                    for k, ncs in enumerate(neg_cs):
                        nc.scalar.activation(
                            out=scr2[:, :CHUNK],
                            in_=psum,
                            func=mybir.ActivationFunctionType.Exp,
                            scale=float(ncs),
                            accum_out=acc[:, q * K + k : q * K + k + 1],
                        )
                    continue

                bi, pj, pk = split
                if q >= len(chunks) - 2:
                    # tail rebalance: ACT takes c_k back so VectorE's
                    # trailing chains don't outlive the last ACT work
                    pk = None
                # Pair adjacent same-weight chunks: both bases land in one
                # [P, 4096] tile and the DVE chain runs once at FD 4096.
                # Its accum goes to the even chunk's slots; the odd slots
                # stay zero (memset) so the host weighting is unchanged.
                can_pair = (
                    kind == "mm"
                    and pend is None
                    and q + 1 < len(chunks)
                    and chunks[q + 1][0] == "mm"
                    and chunks[q + 1][4] == _w
                )
                if can_pair:
                    bpair = scrp.tile([P, 2 * CHUNK], b16, tag="base")
                    half = slice(0, CHUNK)
                    pend = (bpair, q)
                elif pend is not None:
                    bpair = pend[0]
                    half = slice(CHUNK, 2 * CHUNK)
                else:
                    bpair = scrp.tile([P, 2 * CHUNK], b16, tag="base")
                    half = slice(0, CHUNK)
                # base term first so DVE can overlap the rest
                nc.scalar.activation(
                    out=bpair[:, half], in_=psum,
                    func=mybir.ActivationFunctionType.Exp,
                    scale=float(neg_cs[bi]),
                    accum_out=acc[:, q * K + bi : q * K + bi + 1],
                )
                if can_pair:
                    pass  # chain emitted after the partner chunk's base
                elif pend is not None:
                    q0 = pend[1]
                    emit_dve_chain(
                        bpair[:, : 2 * CHUNK], 2 * CHUNK,
                        acc[:, q0 * K + pj : q0 * K + pj + 1],
                        None if pk is None
                        else acc[:, q0 * K + pk : q0 * K + pk + 1],
                    )
                    pend = None
                else:
                    emit_dve_chain(
                        bpair[:, :CHUNK], CHUNK,
                        acc[:, q * K + pj : q * K + pj + 1],
                        None if pk is None
                        else acc[:, q * K + pk : q * K + pk + 1],
                    )
                scr2 = scrp.tile([P, 2 * CHUNK], b16, tag="scr2")
                for k, ncs in enumerate(neg_cs):
                    if k in (bi, pj, pk):
                        continue
                    nc.scalar.activation(
                        out=scr2[:, :CHUNK], in_=psum,
                        func=mybir.ActivationFunctionType.Exp,
                        scale=float(ncs),
                        accum_out=acc[:, q * K + k : q * K + k + 1],
                    )

            nc.sync.dma_start(out=d_acc, in_=acc)

    nc.compile()
    return nc


# ---------------------------------------------------------------- host


def _split_hi_lo(v64):
    hi = v64.astype(bf16)
    lo = (v64 - hi.astype(np.float64)).astype(bf16)
    return hi, lo


def _build_core_inputs(xT_b, yT_b, xnorm, ynorm, core):
    """Per-core input dict. xT_b/yT_b: [D, N] bf16; norms f64 [N]."""
    shift = P * (core + 1)
    rx = np.roll(xT_b, -shift, axis=1)
    ry = np.roll(yT_b, -shift, axis=1)
    ones = np.ones(N, bf16)
    xh, xl = _split_hi_lo(np.roll(xnorm, -shift))
    yh, yl = _split_hi_lo(np.roll(ynorm, -shift))
    raugx = np.stack([ones, ones, xh, xl])
    raugy = np.stack([ones, ones, yh, yl])

    lhs = np.empty((D, 16 * P), bf16)
    laug = np.empty((4, 16 * P), bf16)
    one128 = np.ones(P, bf16)
    for t in range(16):
        r = 8 * (t % 8) + core
        rows = slice(P * r, P * r + P)
        src = xT_b if t < 8 else yT_b
        nsrc = xnorm if t < 8 else ynorm
        lhs[:, P * t : P * (t + 1)] = (
            -2.0 * src[:, rows].astype(np.float32)
        ).astype(bf16)
        nh, nl = _split_hi_lo(nsrc[rows])
        laug[:, P * t : P * (t + 1)] = np.stack([nh, nl, one128, one128])

    mask = np.zeros((P, CHUNK), bf16)
    for s in range(16):
        mask[np.arange(P), P * s + np.arange(P)] = bf16(BIG)

    return {
        "lhs0": np.ascontiguousarray(lhs[:P]),
        "lhs1": np.ascontiguousarray(lhs[P:]),
        "laug": np.ascontiguousarray(laug),
        "rx0": np.ascontiguousarray(rx[:P]),
        "rx1": np.ascontiguousarray(rx[P:]),
        "ry0": np.ascontiguousarray(ry[:P]),
        "ry1": np.ascontiguousarray(ry[P:]),
        "raugx": np.ascontiguousarray(raugx),
        "raugy": np.ascontiguousarray(raugy),
        "maskd": mask,
    }


_NC_CACHE = {}
_WARM = [False]


def _warmup():
    """Run a trivial NEFF once per process: the first NEFF execution in
    an axon session pays ~95 us of ring/queue init that would otherwise
    land inside the measured kernel."""
    if _WARM[0]:
        return
    nc = bacc.Bacc("TRN2", debug=False, enable_asserts=False, num_devices=NCORES)
    f32 = mybir.dt.float32
    d_in = nc.dram_tensor("wx", [P, P], f32, kind="ExternalInput").ap()
    d_out = nc.dram_tensor("wy", [P, P], f32, kind="ExternalOutput").ap()
    with tile.TileContext(nc) as tc:
        with tc.tile_pool(name="pool", bufs=1) as pool:
            t = pool.tile([P, P], f32)
            nc.sync.dma_start(out=t, in_=d_in)
            nc.sync.dma_start(out=d_out, in_=t)
    nc.compile()
    x = np.zeros((P, P), np.float32)
    for attempt in range(3):
        try:
            run_bass_kernel_spmd(
                nc, [{"wx": x}] * NCORES, core_ids=list(range(NCORES))
            )
            break
        except Exception:
            if attempt == 2:
                raise
            import time

            time.sleep(10)
    _WARM[0] = True


def _get_kernel(neg_cs, use_split=True):
    split = pick_split([-v for v in neg_cs]) if use_split else None
    key = (tuple(float(v) for v in neg_cs), split)
    if key not in _NC_CACHE:
        _NC_CACHE[key] = build_kernel(neg_cs, split=split)
    return _NC_CACHE[key]


def _run(source_features, target_features, bandwidths, trace=False, use_split=True):
    x = np.asarray(source_features, np.float32)
    y = np.asarray(target_features, np.float32)
    b = np.asarray(bandwidths, np.float64)
    cs = 1.0 / (2.0 * b * b)
    K = len(cs)
    neg_cs = [-float(c) for c in cs]

    xT_b = np.ascontiguousarray(x.T).astype(bf16)
    yT_b = np.ascontiguousarray(y.T).astype(bf16)
    xnorm = (x.astype(np.float64) ** 2).sum(1)
    ynorm = (y.astype(np.float64) ** 2).sum(1)

    nc = _get_kernel(neg_cs, use_split=use_split)
    in_maps = [
        _build_core_inputs(xT_b, yT_b, xnorm, ynorm, c) for c in range(NCORES)
    ]
    _warmup()
    res = None
    for attempt in range(3):
        try:
            res = run_bass_kernel_spmd(
                nc, in_maps, core_ids=list(range(NCORES)), trace=trace
            )
            break
        except Exception:
            # transient device wedge (NRT_EXEC_UNIT_UNRECOVERABLE) clears
            # on a subsequent attempt; give it a moment and retry
            if attempt == 2:
                raise
            import time

            time.sleep(15)

    weights = np.array([w for (_, _, _, _, w, _) in chunk_list()], np.float64)
    total = 0.0
    for core in range(NCORES):
        a = res.results[core]["acc"].astype(np.float64)  # [P, NCHUNKS*K]
        per_chunk = a.sum(0).reshape(NCHUNKS, K).sum(1)
        total += float(per_chunk @ weights)
    total += 2.0 * N * K  # analytic masked diagonals of kss + ktt
    out = np.float32(total / (float(N) * float(N) * K))
    return np.array(out, dtype=np.float32), res


def kernel(source_features, target_features, bandwidths):
    out, _ = _run(source_features, target_features, bandwidths)
    return out
